# revision 57
# baseline (speedup 1.0000x reference)
"""BiasedAxialAttention (row-attention path) distributed over 8 TRN2 NeuronCores.

Sharding: outer (non-attended) L axis "n" (= p axis 1 after the reference's
permute, = pair axis 2) split into 8 slices of 48 rows.

v2 layout (B=1, L=384, D=128, H=4, DH=32):
  phase 1: LN(x_qkv) -> q,k projections in shuffled [(k*4+s), i] layout via
           scatter-column weights, v in [j, (h,d)] layout, logits over 12
           groups at K=128, stores to DRAM. Inputs are bf16 (host cast).
  window:  ReduceScatter(logits over i) overlapped with the bias->b path and
           the WHOLE gate pipeline (LN + Wg proj, pre-act stored); softmax on
           the i-shard; AllGather(attn) overlapped with bulk Sigmoid.
  phase 3: transpose attn -> AV (per-head 32-row strips), gate multiply
           (+folded cv), Wo with batched psum + fused +bo eviction, DMA out.
"""

import math

import numpy as np
import ml_dtypes

H, DH, D, L = 4, 32, 128, 384
NCORES = 8
R = L // NCORES  # 48
SCALING = 1.0 / math.sqrt(DH)
KSCALE = 1.0 / math.sqrt(L)
EPS = 1e-5
NG = R // 4  # 12 groups of 4 n-rows

_CACHE = {}


def _build_graph():
    import concourse.bass as bass
    import concourse.tile as tile
    from concourse import bacc, mybir

    f32 = mybir.dt.float32
    bf16 = mybir.dt.bfloat16
    Exp = mybir.ActivationFunctionType.Exp
    Identity = mybir.ActivationFunctionType.Identity
    Sigmoid = mybir.ActivationFunctionType.Sigmoid
    Sqrt = mybir.ActivationFunctionType.Sqrt
    sub = mybir.AluOpType.subtract
    mult = mybir.AluOpType.mult
    add = mybir.AluOpType.add

    nc = bacc.Bacc(
        "TRN2", target_bir_lowering=False, debug=False, num_devices=NCORES
    )

    # ---- external params (per-core shards + shared prepped weights) ----
    # x tensors are host-relaid per 4-row group as [NG, 128, 6*256]:
    # six 256-wide pair-interleaved stat blocks (see _prep_host).
    x_qkv = nc.declare_dram_parameter("x_qkv", [NG, D, 6 * 256], bf16, isOutput=False)
    x_gate = nc.declare_dram_parameter("x_gate", [NG, D, 6 * 256], bf16, isOutput=False)
    # bias relaid as [NG, 128, (s t d)] contiguous slabs
    bias_c = nc.declare_dram_parameter("bias_c", [NG, D, 4 * 3 * D], bf16, isOutput=False)
    wq_scat = nc.declare_dram_parameter("wq_scat", [16, D, D], bf16, isOutput=False)
    wk_scat = nc.declare_dram_parameter("wk_scat", [16, D, D], bf16, isOutput=False)
    wv_t = nc.declare_dram_parameter("wv_t", [D, D], bf16, isOutput=False)
    wg_t = nc.declare_dram_parameter("wg_t", [D, D], bf16, isOutput=False)
    wo_t = nc.declare_dram_parameter("wo_t", [D, D], bf16, isOutput=False)
    wb_t = nc.declare_dram_parameter("wb_t", [D, H], bf16, isOutput=False)
    cq_sh = nc.declare_dram_parameter("cq_sh", [H, D], f32, isOutput=False)
    ck_sh = nc.declare_dram_parameter("ck_sh", [H, D], f32, isOutput=False)
    cg_v = nc.declare_dram_parameter("cg_v", [D], f32, isOutput=False)
    cv_v = nc.declare_dram_parameter("cv_v", [D], f32, isOutput=False)
    bo_v = nc.declare_dram_parameter("bo_v", [D], f32, isOutput=False)
    # out in per-group slab layout [NG, 128, (s t d)] bf16; host un-permutes
    out_p = nc.declare_dram_parameter("out", [NG, D, 4 * 3 * D], bf16, isOutput=True)

    # ---- internal DRAM (collective bounces; outs must be Shared) ----
    logits_dram = nc.dram_tensor("logits_dram", [L, H, L], bf16)
    rs_out = nc.dram_tensor("rs_out", [R, H, L], bf16)
    attn_bounce = nc.dram_tensor("attn_bounce", [R, H, L], bf16)
    attn_full = nc.dram_tensor("attn_full", [L, H, L], bf16, addr_space="Shared")
    groups = [list(range(NCORES))]

    with tile.TileContext(nc) as tc:
        from contextlib import ExitStack

        with ExitStack() as top:
            consts = top.enter_context(tc.tile_pool(name="consts", bufs=1))

            # constant tiles
            id_bf = consts.tile([D, D], bf16)
            id_f32 = consts.tile([D, D], f32)
            wqs_sb = consts.tile([D, 16, D], bf16)   # [d, (h,s), P]
            wks_sb = consts.tile([D, 16, D], bf16)
            wv_sb = consts.tile([D, D], bf16)
            wg_sb = consts.tile([D, D], bf16)
            wo_sb = consts.tile([D, D], bf16)
            wb_sb = consts.tile([D, H], bf16)
            cq_sb = consts.tile([D, H], f32)         # per-partition bias, col h
            ck_sb = consts.tile([D, H], f32)
            cg_sb = consts.tile([D, 1], f32)
            cv_sb = consts.tile([D, 1], f32)
            bo_bc3 = consts.tile([D, 3, D], f32)     # bo bcast over partitions+t
            eps128_sb = consts.tile([D, 1], f32)     # 128*eps for folded-var sqrt

            from concourse.masks import make_identity

            make_identity(nc, id_bf)
            make_identity(nc, id_f32)
            nc.sync.dma_start(out=wqs_sb, in_=wq_scat.ap().rearrange("s d p -> d s p"))
            nc.sync.dma_start(out=wks_sb, in_=wk_scat.ap().rearrange("s d p -> d s p"))
            nc.sync.dma_start(out=wv_sb, in_=wv_t[:, :])
            nc.sync.dma_start(out=wg_sb, in_=wg_t[:, :])
            nc.sync.dma_start(out=wo_sb, in_=wo_t[:, :])
            nc.sync.dma_start(out=wb_sb, in_=wb_t[:, :])
            nc.sync.dma_start(out=cq_sb, in_=cq_sh.ap().rearrange("h d -> d h"))
            nc.sync.dma_start(out=ck_sb, in_=ck_sh.ap().rearrange("h d -> d h"))
            nc.sync.dma_start(out=cg_sb, in_=cg_v.ap().unsqueeze(1))
            nc.sync.dma_start(out=cv_sb, in_=cv_v.ap().unsqueeze(1))
            nc.sync.dma_start(
                out=bo_bc3,
                in_=bo_v.ap().unsqueeze(0).unsqueeze(0).broadcast_to((D, 3, D)),
            )
            nc.vector.memset(eps128_sb, D * EPS)

            # persistent stores
            stores = top.enter_context(tc.tile_pool(name="stores", bufs=1))
            v_st = stores.tile([D, 3, R, D], bf16)      # [j, jc, y, (h,d)]
            g_st = stores.tile([D, R, L], bf16)         # [(h,d), y, x]
            b_st = stores.tile([D, 3, H, R], f32)       # [j-part, jc, h, i]
            gst_st = stores.tile([D, NG, 6, 8], f32)    # gate LN stats

            # (s, t) -> (block, half) in the host pair-interleaved layout
            ST2BLK = {
                (0, 0): (0, 0), (0, 1): (0, 1), (0, 2): (1, 0),
                (1, 2): (1, 1), (1, 0): (2, 0), (1, 1): (2, 1),
                (2, 0): (3, 0), (2, 1): (3, 1), (2, 2): (4, 0),
                (3, 2): (4, 1), (3, 0): (5, 0), (3, 1): (5, 1),
            }

            def ln_rows6(xin6, st_p, xh4):
                """LN a 4-row group from the pair-interleaved [128,6,256] tile.

                Each 256-block holds two (row,t) d-vectors interleaved, so one
                bn_stats yields both exact means/vars directly (even/odd
                split == the two vectors). istd is computed unscaled
                (1/(sqrt(128)*sigma)); the sqrt(128) is folded into the
                projection weights host-side.
                """
                stt = st_p.tile([D, 6, 8], f32, tag="st")
                for blk in range(6):
                    nc.vector.bn_stats(
                        out=stt[:, blk, 0:6], in_=xin6[:, blk]
                    )
                # sigma*sqrt(128) = sqrt(cv + 128*eps) over cv slots {2,5}
                nc.scalar.activation(
                    out=stt[:, :, 2:6:3], in_=stt[:, :, 2:6:3], func=Sqrt,
                    bias=eps128_sb, scale=1.0,
                )
                nc.vector.reciprocal(
                    out=stt[:, :, 2:6:3], in_=stt[:, :, 2:6:3]
                )
                for s in range(4):
                    for t in range(3):
                        blk, half = ST2BLK[(s, t)]
                        nc.gpsimd.tensor_scalar(
                            out=xh4[:, s, t],
                            in0=xin6[:, blk, half : 256 : 2],
                            scalar1=stt[:, blk, 1 + 3 * half : 2 + 3 * half],
                            scalar2=stt[:, blk, 2 + 3 * half : 3 + 3 * half],
                            op0=sub,
                            op1=mult,
                        )

            # ---------------- phase 1: LN + QKV + logits ----------------
            qk_ctx = ExitStack()
            qk_st = qk_ctx.enter_context(tc.tile_pool(name="qk_st", bufs=1))
            qsh = qk_st.tile([D, H, NG, L], bf16)   # [(k,s), h, g, i]
            ksh = qk_st.tile([D, H, NG, L], bf16)
            with ExitStack() as ph1:
                xin_p = ph1.enter_context(tc.tile_pool(name="xin", bufs=6))
                st_p = ph1.enter_context(tc.tile_pool(name="stats", bufs=6))
                xh_p = ph1.enter_context(tc.tile_pool(name="xh", bufs=4))
                xt_p = ph1.enter_context(
                    tc.tile_pool(name="xt", bufs=2, space="PSUM")
                )
                xts_p = ph1.enter_context(tc.tile_pool(name="xts", bufs=8))
                vps_p = ph1.enter_context(
                    tc.tile_pool(name="vps", bufs=1, space="PSUM")
                )
                slab_p = ph1.enter_context(
                    tc.tile_pool(name="slab", bufs=5, space="PSUM")
                )

                for g in range(NG):
                    xin6 = xin_p.tile([D, 6, 256], bf16, tag="xin")
                    nc.gpsimd.dma_start(
                        out=xin6.rearrange("p b c -> p (b c)"),
                        in_=x_qkv[g, :, :],
                    )
                    xh4 = xh_p.tile([D, 4, 3, D], bf16, tag="xh")
                    ln_rows6(xin6, st_p, xh4)
                    xts_g = []
                    psq = [slab_p.tile([D, L], f32, tag="slab", name=f"psq_{g}_{h}") for h in range(H)]
                    for s in range(4):
                        n = 4 * g + s
                        xt = xt_p.tile([D, L], bf16, tag="xt")
                        for t in range(3):
                            nc.tensor.transpose(
                                out=xt[:, t * D : (t + 1) * D], in_=xh4[:, s, t],
                                identity=id_bf,
                            )
                        xts3 = xts_p.tile([D, 3, D], bf16, tag="xts",
                                          name=f"xts_{g}_{s}")
                        if s % 2 == 0:
                            nc.vector.tensor_copy(
                                out=xts3.rearrange("p t d -> p (t d)"), in_=xt
                            )
                        else:
                            nc.scalar.activation(
                                out=xts3.rearrange("p t d -> p (t d)"),
                                in_=xt, func=Identity,
                            )
                        xts = xts3.rearrange("p t d -> p (t d)")
                        xts_g.append(xts)
                        # v projection: [j-chunk, (h,d)] x3 into one psum bank
                        vps = vps_p.tile([D, 3, D], f32, tag="vps")
                        for jc in range(3):
                            nc.tensor.matmul(
                                vps[:, jc, :],
                                xts3[:, jc, :],
                                wv_sb,
                                start=True,
                                stop=True,
                            )
                        nc.vector.tensor_copy(out=v_st[:, :, n, :], in_=vps)
                        # q scattered projections accumulate into 4 head slabs
                        for h in range(H):
                            nc.tensor.matmul(
                                psq[h],
                                wqs_sb[:, h * 4 + s, :],
                                xts,
                                start=(s == 0),
                                stop=(s == 3),
                            )
                    for h in range(H):
                        nc.scalar.activation(
                            out=qsh[:, h, g, :], in_=psq[h], func=Identity,
                            bias=cq_sb[:, h : h + 1], scale=1.0,
                        )
                    psk = [slab_p.tile([D, L], f32, tag="slab", name=f"psk_{g}_{h}") for h in range(H)]
                    for s in range(4):
                        for h in range(H):
                            nc.tensor.matmul(
                                psk[h],
                                wks_sb[:, h * 4 + s, :],
                                xts_g[s],
                                start=(s == 0),
                                stop=(s == 3),
                            )
                    for h in range(H):
                        nc.scalar.activation(
                            out=ksh[:, h, g, :], in_=psk[h], func=Identity,
                            bias=ck_sb[:, h : h + 1], scale=1.0,
                        )

                # ---- gate LN stats only -> gst_st (DVE work packed into
                # phase-1 slack). The rows are re-loaded + normalized in the
                # collective window where Pool/PE are idle.
                for g in range(NG):
                    gxin6 = xin_p.tile([D, 6, 256], bf16, tag="gxin")
                    nc.gpsimd.dma_start(
                        out=gxin6.rearrange("p b c -> p (b c)"),
                        in_=x_gate[g, :, :],
                    )
                    for blk in range(6):
                        nc.vector.bn_stats(
                            out=gst_st[:, g, blk, 0:6], in_=gxin6[:, blk]
                        )
                    nc.scalar.activation(
                        out=gst_st[:, g, :, 2:6:3], in_=gst_st[:, g, :, 2:6:3],
                        func=Sqrt, bias=eps128_sb, scale=1.0,
                    )
                    nc.vector.reciprocal(
                        out=gst_st[:, g, :, 2:6:3], in_=gst_st[:, g, :, 2:6:3]
                    )

                # ---- bias -> b path (fills the logits phase) ----
                bt_p = ph1.enter_context(tc.tile_pool(name="bt", bufs=2))
                bts_p = ph1.enter_context(tc.tile_pool(name="bts", bufs=3))
                for g in range(NG):
                    bin4 = bt_p.tile([D, 4, 3, D], bf16, tag="bin")
                    nc.gpsimd.dma_start(
                        out=bin4.rearrange("p s t d -> p (s t d)"),
                        in_=bias_c[g, :, :],
                    )
                    for s4 in range(4):
                        i_row = 4 * g + s4
                        btp = xt_p.tile([D, L], bf16, tag="xt",
                                        name=f"btp_{g}_{s4}")
                        for t in range(3):
                            nc.tensor.transpose(
                                out=btp[:, t * D : (t + 1) * D],
                                in_=bin4[:, s4, t],
                                identity=id_bf,
                            )
                        bts = bts_p.tile([D, 3, D], bf16, tag="bts",
                                         name=f"bts_{g}_{s4}")
                        nc.vector.tensor_copy(
                            out=bts.rearrange("p t d -> p (t d)"), in_=btp
                        )
                        bpp = vps_p.tile([D, 3, H], f32, tag="vps",
                                         name=f"bpp_{g}_{s4}")
                        for t in range(3):
                            nc.tensor.matmul(
                                bpp[:, t, :],
                                bts[:, t, :],
                                wb_sb,
                                start=True,
                                stop=True,
                            )
                        nc.vector.tensor_copy(
                            out=b_st[:, :, :, i_row], in_=bpp
                        )

            # logits: [i-chunk, j] per head, K=128 over 12 groups
            with ExitStack() as phl:
                lg_p = phl.enter_context(
                    tc.tile_pool(name="lgp", bufs=2, space="PSUM")
                )
                ls_p = phl.enter_context(tc.tile_pool(name="lsb", bufs=3))
                for ic in range(3):
                    lsb = ls_p.tile([D, H, L], bf16, tag="lsb")
                    for h in range(H):
                        pl = lg_p.tile([D, L], f32, tag="lg")
                        for g in range(NG):
                            nc.tensor.matmul(
                                pl,
                                qsh[:, h, g, ic * D : (ic + 1) * D],
                                ksh[:, h, g, :],
                                start=(g == 0),
                                stop=(g == NG - 1),
                            )
                        nc.scalar.activation(
                            out=lsb[:, h, :], in_=pl, func=Identity
                        )
                    ldst = bass.AP(
                        tensor=logits_dram.ap().tensor,
                        offset=ic * H * L,
                        ap=[[3 * H * L, D], [1, H * L]],
                    )
                    nc.scalar.dma_start(
                        out=ldst, in_=lsb.rearrange("p h l -> p (h l)")
                    )

            qk_ctx.close()

            # ---------------- collective 1: ReduceScatter over i ----------------
            nc.gpsimd.collective_compute(
                "ReduceScatter",
                add,
                replica_groups=groups,
                ins=[logits_dram.ap().opt()],
                outs=[rs_out.ap().opt()],
            )

            # ------- collective window: softmax + gate pipeline -------
            with ExitStack() as phw:
                # ---- softmax on the i-shard ----
                sm_p = phw.enter_context(tc.tile_pool(name="sm", bufs=1))
                smp_p = phw.enter_context(
                    tc.tile_pool(name="smp", bufs=2, space="PSUM")
                )
                rs_sb = sm_p.tile([R, H, L], bf16)
                b2_sb = sm_p.tile([R, H, 3, D], f32)
                nc.scalar.dma_start(
                    out=rs_sb.rearrange("i h l -> i (h l)"),
                    in_=rs_out.ap().rearrange("i h l -> i (h l)"),
                )
                for h in range(H):
                    for jc in range(3):
                        btp2 = smp_p.tile([R, D], f32, tag="btp2")
                        nc.tensor.transpose(
                            out=btp2, in_=b_st[:, jc, h, :], identity=id_f32,
                        )
                        nc.vector.tensor_copy(out=b2_sb[:, h, jc, :], in_=btp2)
                ex_in = sm_p.tile([R, H, L], f32)
                nc.vector.tensor_add(
                    out=ex_in,
                    in0=rs_sb,
                    in1=b2_sb.rearrange("i h t d -> i h (t d)"),
                )
                exp_sb = sm_p.tile([R, H, L], f32)
                sums = sm_p.tile([R, H], f32)
                for h in range(H):
                    nc.scalar.activation(
                        out=exp_sb[:, h, :], in_=ex_in[:, h, :], func=Exp,
                        accum_out=sums[:, h : h + 1],
                    )
                rsum = sm_p.tile([R, H], f32)
                nc.vector.reciprocal(out=rsum, in_=sums)
                attn_sb = sm_p.tile([R, H, L], bf16)
                for h in range(H):
                    nc.gpsimd.tensor_scalar(
                        out=attn_sb[:, h, :],
                        in0=exp_sb[:, h, :],
                        scalar1=rsum[:, h : h + 1],
                        scalar2=None,
                        op0=mult,
                    )
                nc.scalar.dma_start(out=attn_bounce[:, :, :], in_=attn_sb)

                # ---- gate re-load + normalize (Pool) + transpose + proj ----
                gxin_p = phw.enter_context(tc.tile_pool(name="gxin2", bufs=3))
                gxh_p = phw.enter_context(tc.tile_pool(name="gxh", bufs=3))
                gxt_p = phw.enter_context(
                    tc.tile_pool(name="gxt", bufs=2, space="PSUM")
                )
                gxts_p = phw.enter_context(tc.tile_pool(name="gxts", bufs=4))
                gp_p = phw.enter_context(
                    tc.tile_pool(name="gp", bufs=2, space="PSUM")
                )
                for g in range(NG):
                    gxin6 = gxin_p.tile([D, 6, 256], bf16, tag="gxin2")
                    nc.sync.dma_start(
                        out=gxin6.rearrange("p b c -> p (b c)"),
                        in_=x_gate[g, :, :],
                    )
                    gxh4 = gxh_p.tile([D, 4, 3, D], bf16, tag="gxh")
                    for s in range(4):
                        for t in range(3):
                            blk, half = ST2BLK[(s, t)]
                            nc.gpsimd.tensor_scalar(
                                out=gxh4[:, s, t],
                                in0=gxin6[:, blk, half : 256 : 2],
                                scalar1=gst_st[:, g, blk,
                                               1 + 3 * half : 2 + 3 * half],
                                scalar2=gst_st[:, g, blk,
                                               2 + 3 * half : 3 + 3 * half],
                                op0=sub,
                                op1=mult,
                            )
                    for s in range(4):
                        y = 4 * g + s
                        gxt = gxt_p.tile([D, L], bf16, tag="gxt")
                        for t in range(3):
                            nc.tensor.transpose(
                                out=gxt[:, t * D : (t + 1) * D],
                                in_=gxh4[:, s, t],
                                identity=id_bf,
                            )
                        gxts = gxts_p.tile([D, L], bf16, tag="gxts")
                        if s % 2 == 0:
                            nc.vector.tensor_copy(out=gxts, in_=gxt)
                        else:
                            nc.scalar.activation(out=gxts, in_=gxt, func=Identity)
                        gp = gp_p.tile([D, L], f32, tag="gp")
                        nc.tensor.matmul(gp, wg_sb, gxts, start=True, stop=True)
                        # pre-activation (Wg xhat + cg) stored; Sigmoid later
                        nc.scalar.activation(
                            out=g_st[:, y, :], in_=gp, func=Identity,
                            bias=cg_sb, scale=1.0,
                        )

            # ---------------- collective 2: AllGather attn ----------------
            nc.gpsimd.collective_compute(
                "AllGather",
                mybir.AluOpType.bypass,
                replica_groups=groups,
                ins=[attn_bounce.ap().opt()],
                outs=[attn_full.ap().opt()],
            )

            # -------- window work C: bulk Sigmoid on gate pre-acts ---------
            for y in range(R):
                nc.scalar.activation(
                    out=g_st[:, y, :], in_=g_st[:, y, :], func=Sigmoid
                )

            # ---------------- phase 3: attn^T, AV, gate, Wo ----------------
            with ExitStack() as ph4:
                at_in_p = ph4.enter_context(tc.tile_pool(name="atin", bufs=3))
                at_st = ph4.enter_context(tc.tile_pool(name="atst", bufs=1))
                attnT = at_st.tile([D, H, 3, L], bf16)  # [j, h, jc, x]
                at_in = [at_in_p.tile([D, H, L], bf16, tag="atin", name=f"at_in_{i}") for i in range(3)]
                for ic in range(3):
                    asrc = bass.AP(
                        tensor=attn_full.ap().tensor,
                        offset=ic * H * L,
                        ap=[[3 * H * L, D], [1, H * L]],
                    )
                    nc.scalar.dma_start(
                        out=at_in[ic].rearrange("p h l -> p (h l)"), in_=asrc
                    )
                for h in range(H):
                    for jc in range(3):
                        for ic in range(3):
                            nc.sync.dma_start_transpose(
                                out=attnT[:, h, jc, ic * D : (ic + 1) * D],
                                in_=at_in[ic][:, h, jc * D : (jc + 1) * D],
                            )

                av_p = ph4.enter_context(
                    tc.tile_pool(name="av", bufs=3, space="PSUM")
                )
                gt_p = ph4.enter_context(tc.tile_pool(name="gt", bufs=4))
                wo_ps = ph4.enter_context(
                    tc.tile_pool(name="wops", bufs=2, space="PSUM")
                )
                os_p = ph4.enter_context(tc.tile_pool(name="osb", bufs=4))
                for y in range(R):
                    pav = av_p.tile([D, L], f32, tag="av")
                    for h in range(H):
                        for jc in range(3):
                            nc.tensor.matmul(
                                pav[h * DH : (h + 1) * DH, :],
                                v_st[:, jc, y, h * DH : (h + 1) * DH],
                                attnT[:, h, jc, :],
                                start=(jc == 0),
                                stop=(jc == 2),
                                tile_position=(0, h * DH),
                            )
                    gated = gt_p.tile([D, L], bf16, tag="gt")
                    nc.vector.scalar_tensor_tensor(
                        out=gated,
                        in0=pav,
                        scalar=cv_sb,
                        in1=g_st[:, y, :],
                        op0=add,
                        op1=mult,
                    )
                    pwo = wo_ps.tile([D, 3, D], f32, tag="wops")
                    for xc in range(3):
                        nc.tensor.matmul(
                            pwo[:, xc, :],
                            gated[:, xc * D : (xc + 1) * D],
                            wo_sb,
                            start=True,
                            stop=True,
                        )
                    if y % 4 == 0:
                        osb4 = os_p.tile([D, 4, 3, D], bf16, tag="osb",
                                         name=f"osb4_{y}")
                    # fused psum->sbuf eviction with +bo
                    nc.vector.tensor_add(
                        out=osb4[:, y % 4], in0=pwo, in1=bo_bc3
                    )
                    if y % 4 == 3:
                        nc.sync.dma_start(
                            out=out_p[y // 4, :, :],
                            in_=osb4.rearrange("p s t d -> p (s t d)"),
                        )

    nc.compile()
    return nc


def _prep_host(inputs):
    """Host-side: shard inputs, fold LN scale/bias + constants into weights."""
    f32 = np.float32
    bf = ml_dtypes.bfloat16
    pair = np.ascontiguousarray(np.asarray(inputs["pair"], f32)[0])
    bias = np.ascontiguousarray(np.asarray(inputs["bias"], f32)[0])
    ln_scale = np.asarray(inputs["ln_scale"], f32)
    ln_bias = np.asarray(inputs["ln_bias"], f32)
    Wq = np.asarray(inputs["Wq"], f32)
    Wk = np.asarray(inputs["Wk"], f32)
    Wv = np.asarray(inputs["Wv"], f32)
    Wb = np.asarray(inputs["Wb"], f32)
    Wg = np.asarray(inputs["Wg"], f32)
    bg = np.asarray(inputs["bg"], f32)
    Wo = np.asarray(inputs["Wo"], f32)
    bo = np.asarray(inputs["bo"], f32)

    RT_D = math.sqrt(D)  # istd is computed unscaled on-chip; fold sqrt(128)
    Wq_eff = Wq * ln_scale[None, :] * SCALING * RT_D
    Wk_eff = Wk * ln_scale[None, :] * KSCALE * RT_D
    cq = (Wq @ ln_bias) * SCALING
    ck = (Wk @ ln_bias) * KSCALE

    def scat(W_eff):
        w = np.zeros((16, D, D), f32)
        for h in range(H):
            for s in range(4):
                for kk in range(DH):
                    w[h * 4 + s, :, kk * 4 + s] = W_eff[h * DH + kk, :]
        return w.astype(bf)

    wq_scat = scat(Wq_eff)
    wk_scat = scat(Wk_eff)
    cq_sh = np.zeros((H, D), f32)
    ck_sh = np.zeros((H, D), f32)
    for h in range(H):
        for s in range(4):
            for kk in range(DH):
                cq_sh[h, kk * 4 + s] = cq[h * DH + kk]
                ck_sh[h, kk * 4 + s] = ck[h * DH + kk]

    pair_bf = pair.astype(bf)
    bias_bf = bias.astype(bf)
    shared = {
        "wq_scat": wq_scat,
        "wk_scat": wk_scat,
        "wv_t": (Wv * ln_scale[None, :] * RT_D).T.astype(bf).copy(),
        "wg_t": (Wg * ln_scale[None, :] * RT_D).T.astype(bf).copy(),
        "wo_t": Wo.T.astype(bf).copy(),
        "wb_t": Wb.T.astype(bf).copy(),
        "cq_sh": cq_sh,
        "ck_sh": ck_sh,
        "cg_v": (Wg @ ln_bias + bg).astype(f32),
        "cv_v": (Wv @ ln_bias).astype(f32),
        "bo_v": bo.astype(f32),
    }

    # pair-interleaved 6-block relayout: [48,384,128] -> [NG,128,6*256]
    BLK = [(0, 0, 0), (0, 1, 0), (0, 2, 1), (1, 2, 1), (1, 0, 2), (1, 1, 2),
           (2, 0, 3), (2, 1, 3), (2, 2, 4), (3, 2, 4), (3, 0, 5), (3, 1, 5)]

    def relayout6(x48):
        tmp = x48.reshape(NG, 4, 128, 3, D)  # [g, s, p, t, d]
        arr = np.empty((NG, 128, 6, D, 2), x48.dtype)
        half_used = [0] * 6
        for s, t, blk in BLK:
            arr[:, :, blk, :, half_used[blk]] = tmp[:, s, :, t, :]
            half_used[blk] += 1
        return np.ascontiguousarray(arr.reshape(NG, 128, 6 * 256))

    def relayout_std(x48):
        tmp = x48.reshape(NG, 4, 128, 3, D)  # [g, s, p, t, d]
        return np.ascontiguousarray(
            tmp.transpose(0, 2, 1, 3, 4).reshape(NG, 128, 4 * 3 * D)
        )

    in_maps = []
    for c in range(NCORES):
        sl = slice(c * R, (c + 1) * R)
        m = dict(shared)
        m["x_qkv"] = relayout6(
            np.ascontiguousarray(pair_bf[:, sl, :].transpose(1, 0, 2))
        )
        m["x_gate"] = relayout6(np.ascontiguousarray(pair_bf[sl, :, :]))
        m["bias_c"] = relayout_std(np.ascontiguousarray(bias_bf[sl, :, :]))
        in_maps.append(m)
    return in_maps


def kernel(**inputs):
    import os
    from concourse.bass_utils import run_bass_kernel_spmd

    in_maps = _prep_host(inputs)
    if "nc" not in _CACHE:
        _CACHE["nc"] = _build_graph()
    nc = _CACHE["nc"]
    kw = {}
    if os.environ.get("BAX_TRACE"):
        kw = dict(trace=True, tmpdir=os.environ.get("BAX_TRACE_DIR") or None)
    res = run_bass_kernel_spmd(nc, in_maps, list(range(NCORES)), **kw)
    _CACHE["last_result"] = res
    out = np.zeros((1, L, L, D), np.float32)
    for c in range(NCORES):
        # [NG,128,(s t d)] bf16 -> [R, L, D] f32 (x = 3p + t)
        o = np.asarray(res.results[c]["out"], np.float32)
        o = o.reshape(NG, 128, 4, 3, D).transpose(0, 2, 1, 3, 4)
        out[0, c * R : (c + 1) * R, :, :] = o.reshape(R, L, D)
    return out


if __name__ == "__main__":
    nc = _build_graph()
    print("graph built ok")


# revision 60
# speedup vs baseline: 1.0511x; 1.0511x over previous
"""BiasedAxialAttention (row-attention path) distributed over 8 TRN2 NeuronCores.

Sharding: outer (non-attended) L axis "n" (= p axis 1 after the reference's
permute, = pair axis 2) split into 8 slices of 48 rows.

v2 layout (B=1, L=384, D=128, H=4, DH=32):
  phase 1: LN(x_qkv) -> q,k projections in shuffled [(k*4+s), i] layout via
           scatter-column weights, v in [j, (h,d)] layout, logits over 12
           groups at K=128, stores to DRAM. Inputs are bf16 (host cast).
  window:  ReduceScatter(logits over i) overlapped with the bias->b path and
           the WHOLE gate pipeline (LN + Wg proj, pre-act stored); softmax on
           the i-shard; AllGather(attn) overlapped with bulk Sigmoid.
  phase 3: transpose attn -> AV (per-head 32-row strips), gate multiply
           (+folded cv), Wo with batched psum + fused +bo eviction, DMA out.
"""

import math

import numpy as np
import ml_dtypes

H, DH, D, L = 4, 32, 128, 384
NCORES = 8
R = L // NCORES  # 48
SCALING = 1.0 / math.sqrt(DH)
KSCALE = 1.0 / math.sqrt(L)
EPS = 1e-5
NG = R // 4  # 12 groups of 4 n-rows

_CACHE = {}


def _build_graph():
    import concourse.bass as bass
    import concourse.tile as tile
    from concourse import bacc, mybir

    f32 = mybir.dt.float32
    bf16 = mybir.dt.bfloat16
    Exp = mybir.ActivationFunctionType.Exp
    Identity = mybir.ActivationFunctionType.Identity
    Sigmoid = mybir.ActivationFunctionType.Sigmoid
    Sqrt = mybir.ActivationFunctionType.Sqrt
    sub = mybir.AluOpType.subtract
    mult = mybir.AluOpType.mult
    add = mybir.AluOpType.add

    nc = bacc.Bacc(
        "TRN2", target_bir_lowering=False, debug=False, num_devices=NCORES
    )

    # ---- external params (per-core shards + shared prepped weights) ----
    # x tensors are host-relaid per 4-row group as [NG, 128, 6*256]:
    # six 256-wide pair-interleaved stat blocks (see _prep_host).
    x_qkv = nc.declare_dram_parameter("x_qkv", [NG, D, 6 * 256], bf16, isOutput=False)
    x_gate = nc.declare_dram_parameter("x_gate", [NG, D, 6 * 256], bf16, isOutput=False)
    # bias relaid as [NG, 128, (s t d)] contiguous slabs
    bias_c = nc.declare_dram_parameter("bias_c", [NG, D, 4 * 3 * D], bf16, isOutput=False)
    wq_scat = nc.declare_dram_parameter("wq_scat", [16, D, D], bf16, isOutput=False)
    wk_scat = nc.declare_dram_parameter("wk_scat", [16, D, D], bf16, isOutput=False)
    wv_t = nc.declare_dram_parameter("wv_t", [D, D], bf16, isOutput=False)
    wg_t = nc.declare_dram_parameter("wg_t", [D, D], bf16, isOutput=False)
    wo_t = nc.declare_dram_parameter("wo_t", [D, D], bf16, isOutput=False)
    wb_t = nc.declare_dram_parameter("wb_t", [D, H], bf16, isOutput=False)
    cq_sh = nc.declare_dram_parameter("cq_sh", [H, D], f32, isOutput=False)
    ck_sh = nc.declare_dram_parameter("ck_sh", [H, D], f32, isOutput=False)
    cg_v = nc.declare_dram_parameter("cg_v", [D], f32, isOutput=False)
    cv_v = nc.declare_dram_parameter("cv_v", [D], f32, isOutput=False)
    bo_v = nc.declare_dram_parameter("bo_v", [D], f32, isOutput=False)
    # out in per-group slab layout [NG, 128, (s t d)] bf16; host un-permutes
    out_p = nc.declare_dram_parameter("out", [NG, D, 4 * 3 * D], bf16, isOutput=True)

    # ---- internal DRAM (collective bounces; outs must be Shared) ----
    logits_dram = nc.dram_tensor("logits_dram", [L, H, L], bf16)
    rs_out = nc.dram_tensor("rs_out", [R, H, L], bf16)
    attn_bounce = nc.dram_tensor("attn_bounce", [R, H, L], bf16)
    attn_full = nc.dram_tensor("attn_full", [L, H, L], bf16, addr_space="Shared")
    groups = [list(range(NCORES))]

    with tile.TileContext(nc) as tc:
        from contextlib import ExitStack

        with ExitStack() as top:
            consts = top.enter_context(tc.tile_pool(name="consts", bufs=1))

            # constant tiles
            id_bf = consts.tile([D, D], bf16)
            id_f32 = consts.tile([D, D], f32)
            wqs_sb = consts.tile([D, 16, D], bf16)   # [d, (h,s), P]
            wks_sb = consts.tile([D, 16, D], bf16)
            wv_sb = consts.tile([D, D], bf16)
            wg_sb = consts.tile([D, D], bf16)
            wo_sb = consts.tile([D, D], bf16)
            wb_sb = consts.tile([D, H], bf16)
            cq_sb = consts.tile([D, H], f32)         # per-partition bias, col h
            ck_sb = consts.tile([D, H], f32)
            cg_sb = consts.tile([D, 1], f32)
            cv_sb = consts.tile([D, 1], f32)
            bo_bc3 = consts.tile([D, 3, D], f32)     # bo bcast over partitions+t
            eps128_sb = consts.tile([D, 1], f32)     # 128*eps for folded-var sqrt

            from concourse.masks import make_identity

            make_identity(nc, id_bf)
            make_identity(nc, id_f32)
            nc.sync.dma_start(out=wqs_sb, in_=wq_scat.ap().rearrange("s d p -> d s p"))
            nc.sync.dma_start(out=wks_sb, in_=wk_scat.ap().rearrange("s d p -> d s p"))
            nc.sync.dma_start(out=wv_sb, in_=wv_t[:, :])
            nc.sync.dma_start(out=wg_sb, in_=wg_t[:, :])
            nc.sync.dma_start(out=wo_sb, in_=wo_t[:, :])
            nc.sync.dma_start(out=wb_sb, in_=wb_t[:, :])
            nc.sync.dma_start(out=cq_sb, in_=cq_sh.ap().rearrange("h d -> d h"))
            nc.sync.dma_start(out=ck_sb, in_=ck_sh.ap().rearrange("h d -> d h"))
            nc.sync.dma_start(out=cg_sb, in_=cg_v.ap().unsqueeze(1))
            nc.sync.dma_start(out=cv_sb, in_=cv_v.ap().unsqueeze(1))
            nc.sync.dma_start(
                out=bo_bc3,
                in_=bo_v.ap().unsqueeze(0).unsqueeze(0).broadcast_to((D, 3, D)),
            )
            nc.vector.memset(eps128_sb, D * EPS)

            # persistent stores
            stores = top.enter_context(tc.tile_pool(name="stores", bufs=1))
            v_st = stores.tile([D, 3, R, D], bf16)      # [j, jc, y, (h,d)]
            g_st = stores.tile([D, R, L], bf16)         # [(h,d), y, x]
            b_st = stores.tile([D, 3, H, R], f32)       # [j-part, jc, h, i]
            gst_st = stores.tile([D, NG, 6, 8], f32)    # gate LN stats

            # (s, t) -> (block, half) in the host pair-interleaved layout
            ST2BLK = {
                (0, 0): (0, 0), (0, 1): (0, 1), (0, 2): (1, 0),
                (1, 2): (1, 1), (1, 0): (2, 0), (1, 1): (2, 1),
                (2, 0): (3, 0), (2, 1): (3, 1), (2, 2): (4, 0),
                (3, 2): (4, 1), (3, 0): (5, 0), (3, 1): (5, 1),
            }

            def ln_rows6(xin6, st_p, xh4):
                """LN a 4-row group from the pair-interleaved [128,6,256] tile.

                Each 256-block holds two (row,t) d-vectors interleaved, so one
                bn_stats yields both exact means/vars directly (even/odd
                split == the two vectors). istd is computed unscaled
                (1/(sqrt(128)*sigma)); the sqrt(128) is folded into the
                projection weights host-side.
                """
                stt = st_p.tile([D, 6, 8], f32, tag="st")
                for blk in range(6):
                    nc.vector.bn_stats(
                        out=stt[:, blk, 0:6], in_=xin6[:, blk]
                    )
                # sigma*sqrt(128) = sqrt(cv + 128*eps) over cv slots {2,5}
                nc.scalar.activation(
                    out=stt[:, :, 2:6:3], in_=stt[:, :, 2:6:3], func=Sqrt,
                    bias=eps128_sb, scale=1.0,
                )
                nc.vector.reciprocal(
                    out=stt[:, :, 2:6:3], in_=stt[:, :, 2:6:3]
                )
                for s in range(4):
                    for t in range(3):
                        blk, half = ST2BLK[(s, t)]
                        nc.gpsimd.tensor_scalar(
                            out=xh4[:, s, t],
                            in0=xin6[:, blk, half : 256 : 2],
                            scalar1=stt[:, blk, 1 + 3 * half : 2 + 3 * half],
                            scalar2=stt[:, blk, 2 + 3 * half : 3 + 3 * half],
                            op0=sub,
                            op1=mult,
                        )

            # ---------------- phase 1: LN + QKV + logits ----------------
            qk_ctx = ExitStack()
            qk_st = qk_ctx.enter_context(tc.tile_pool(name="qk_st", bufs=1))
            qsh = qk_st.tile([D, H, NG, L], bf16)   # [(k,s), h, g, i]
            ksh = qk_st.tile([D, H, NG, L], bf16)
            with ExitStack() as ph1:
                xin_p = ph1.enter_context(tc.tile_pool(name="xin", bufs=4))
                st_p = ph1.enter_context(tc.tile_pool(name="stats", bufs=4))
                xh_p = ph1.enter_context(tc.tile_pool(name="xh", bufs=3))
                xt_p = ph1.enter_context(
                    tc.tile_pool(name="xt", bufs=2, space="PSUM")
                )
                xts_p = ph1.enter_context(tc.tile_pool(name="xts", bufs=6))
                vps_p = ph1.enter_context(
                    tc.tile_pool(name="vps", bufs=1, space="PSUM")
                )
                slab_p = ph1.enter_context(
                    tc.tile_pool(name="slab", bufs=5, space="PSUM")
                )

                for g in range(NG):
                    xin6 = xin_p.tile([D, 6, 256], bf16, tag="xin")
                    nc.gpsimd.dma_start(
                        out=xin6.rearrange("p b c -> p (b c)"),
                        in_=x_qkv[g, :, :],
                    )
                    xh4 = xh_p.tile([D, 4, 3, D], bf16, tag="xh")
                    ln_rows6(xin6, st_p, xh4)
                    xts_g = []
                    psq = [slab_p.tile([D, L], f32, tag="slab", name=f"psq_{g}_{h}") for h in range(H)]
                    for s in range(4):
                        n = 4 * g + s
                        xt = xt_p.tile([D, L], bf16, tag="xt")
                        for t in range(3):
                            nc.tensor.transpose(
                                out=xt[:, t * D : (t + 1) * D], in_=xh4[:, s, t],
                                identity=id_bf,
                            )
                        xts3 = xts_p.tile([D, 3, D], bf16, tag="xts",
                                          name=f"xts_{g}_{s}")
                        if s % 2 == 0:
                            nc.vector.tensor_copy(
                                out=xts3.rearrange("p t d -> p (t d)"), in_=xt
                            )
                        else:
                            nc.scalar.activation(
                                out=xts3.rearrange("p t d -> p (t d)"),
                                in_=xt, func=Identity,
                            )
                        xts = xts3.rearrange("p t d -> p (t d)")
                        xts_g.append(xts)
                        # v projection: [j-chunk, (h,d)] x3 into one psum bank
                        vps = vps_p.tile([D, 3, D], f32, tag="vps")
                        for jc in range(3):
                            nc.tensor.matmul(
                                vps[:, jc, :],
                                xts3[:, jc, :],
                                wv_sb,
                                start=True,
                                stop=True,
                            )
                        nc.vector.tensor_copy(out=v_st[:, :, n, :], in_=vps)
                        # q scattered projections accumulate into 4 head slabs
                        for h in range(H):
                            nc.tensor.matmul(
                                psq[h],
                                wqs_sb[:, h * 4 + s, :],
                                xts,
                                start=(s == 0),
                                stop=(s == 3),
                            )
                    for h in range(H):
                        nc.scalar.activation(
                            out=qsh[:, h, g, :], in_=psq[h], func=Identity,
                            bias=cq_sb[:, h : h + 1], scale=1.0,
                        )
                    psk = [slab_p.tile([D, L], f32, tag="slab", name=f"psk_{g}_{h}") for h in range(H)]
                    for s in range(4):
                        for h in range(H):
                            nc.tensor.matmul(
                                psk[h],
                                wks_sb[:, h * 4 + s, :],
                                xts_g[s],
                                start=(s == 0),
                                stop=(s == 3),
                            )
                    for h in range(H):
                        nc.scalar.activation(
                            out=ksh[:, h, g, :], in_=psk[h], func=Identity,
                            bias=ck_sb[:, h : h + 1], scale=1.0,
                        )

                # ---- gate LN stats only -> gst_st (DVE work packed into
                # phase-1 slack). The rows are re-loaded + normalized in the
                # collective window where Pool/PE are idle.
                for g in range(NG):
                    gxin6 = xin_p.tile([D, 6, 256], bf16, tag="gxin")
                    nc.gpsimd.dma_start(
                        out=gxin6.rearrange("p b c -> p (b c)"),
                        in_=x_gate[g, :, :],
                    )
                    for blk in range(6):
                        nc.vector.bn_stats(
                            out=gst_st[:, g, blk, 0:6], in_=gxin6[:, blk]
                        )
                    nc.scalar.activation(
                        out=gst_st[:, g, :, 2:6:3], in_=gst_st[:, g, :, 2:6:3],
                        func=Sqrt, bias=eps128_sb, scale=1.0,
                    )
                    nc.vector.reciprocal(
                        out=gst_st[:, g, :, 2:6:3], in_=gst_st[:, g, :, 2:6:3]
                    )

                # ---- bias -> b path (fills the logits phase) ----
                bt_p = ph1.enter_context(tc.tile_pool(name="bt", bufs=2))
                bts_p = ph1.enter_context(tc.tile_pool(name="bts", bufs=3))
                for g in range(NG):
                    bin4 = bt_p.tile([D, 4, 3, D], bf16, tag="bin")
                    nc.gpsimd.dma_start(
                        out=bin4.rearrange("p s t d -> p (s t d)"),
                        in_=bias_c[g, :, :],
                    )
                    for s4 in range(4):
                        i_row = 4 * g + s4
                        btp = xt_p.tile([D, L], bf16, tag="xt",
                                        name=f"btp_{g}_{s4}")
                        for t in range(3):
                            nc.tensor.transpose(
                                out=btp[:, t * D : (t + 1) * D],
                                in_=bin4[:, s4, t],
                                identity=id_bf,
                            )
                        bts = bts_p.tile([D, 3, D], bf16, tag="bts",
                                         name=f"bts_{g}_{s4}")
                        nc.vector.tensor_copy(
                            out=bts.rearrange("p t d -> p (t d)"), in_=btp
                        )
                        bpp = vps_p.tile([D, 3, H], f32, tag="vps",
                                         name=f"bpp_{g}_{s4}")
                        for t in range(3):
                            nc.tensor.matmul(
                                bpp[:, t, :],
                                bts[:, t, :],
                                wb_sb,
                                start=True,
                                stop=True,
                            )
                        nc.vector.tensor_copy(
                            out=b_st[:, :, :, i_row], in_=bpp
                        )

            # logits: [i-chunk, j] per head, K=128 over 12 groups
            with ExitStack() as phl:
                lg_p = phl.enter_context(
                    tc.tile_pool(name="lgp", bufs=2, space="PSUM")
                )
                ls_p = phl.enter_context(tc.tile_pool(name="lsb", bufs=3))
                for ic in range(3):
                    lsb = ls_p.tile([D, H, L], bf16, tag="lsb")
                    for h in range(H):
                        pl = lg_p.tile([D, L], f32, tag="lg")
                        for g in range(NG):
                            nc.tensor.matmul(
                                pl,
                                qsh[:, h, g, ic * D : (ic + 1) * D],
                                ksh[:, h, g, :],
                                start=(g == 0),
                                stop=(g == NG - 1),
                            )
                        nc.scalar.activation(
                            out=lsb[:, h, :], in_=pl, func=Identity
                        )
                    ldst = bass.AP(
                        tensor=logits_dram.ap().tensor,
                        offset=ic * H * L,
                        ap=[[3 * H * L, D], [1, H * L]],
                    )
                    nc.scalar.dma_start(
                        out=ldst, in_=lsb.rearrange("p h l -> p (h l)")
                    )

            qk_ctx.close()

            # ---------------- collective 1: ReduceScatter over i ----------------
            nc.gpsimd.collective_compute(
                "ReduceScatter",
                add,
                replica_groups=groups,
                ins=[logits_dram.ap().opt()],
                outs=[rs_out.ap().opt()],
            )

            # ------- collective window: softmax + gate pipeline -------
            with ExitStack() as phw:
                # ---- softmax on the i-shard ----
                sm_p = phw.enter_context(tc.tile_pool(name="sm", bufs=1))
                smp_p = phw.enter_context(
                    tc.tile_pool(name="smp", bufs=2, space="PSUM")
                )
                rs_sb = sm_p.tile([R, H, L], bf16)
                b2_sb = sm_p.tile([R, H, 3, D], f32)
                nc.scalar.dma_start(
                    out=rs_sb.rearrange("i h l -> i (h l)"),
                    in_=rs_out.ap().rearrange("i h l -> i (h l)"),
                )
                for h in range(H):
                    for jc in range(3):
                        btp2 = smp_p.tile([R, D], f32, tag="btp2")
                        nc.tensor.transpose(
                            out=btp2, in_=b_st[:, jc, h, :], identity=id_f32,
                        )
                        nc.vector.tensor_copy(out=b2_sb[:, h, jc, :], in_=btp2)
                ex_in = sm_p.tile([R, H, L], f32)
                nc.vector.tensor_add(
                    out=ex_in,
                    in0=rs_sb,
                    in1=b2_sb.rearrange("i h t d -> i h (t d)"),
                )
                exp_sb = sm_p.tile([R, H, L], f32)
                sums = sm_p.tile([R, H], f32)
                for h in range(H):
                    nc.scalar.activation(
                        out=exp_sb[:, h, :], in_=ex_in[:, h, :], func=Exp,
                        accum_out=sums[:, h : h + 1],
                    )
                rsum = sm_p.tile([R, H], f32)
                nc.vector.reciprocal(out=rsum, in_=sums)
                attn_sb = sm_p.tile([R, H, L], bf16)
                for h in range(H):
                    nc.gpsimd.tensor_scalar(
                        out=attn_sb[:, h, :],
                        in0=exp_sb[:, h, :],
                        scalar1=rsum[:, h : h + 1],
                        scalar2=None,
                        op0=mult,
                    )
                nc.scalar.dma_start(out=attn_bounce[:, :, :], in_=attn_sb)

                # ---- gate re-load + normalize (Pool) + transpose + proj ----
                gxin_p = phw.enter_context(tc.tile_pool(name="gxin2", bufs=3))
                gxh_p = phw.enter_context(tc.tile_pool(name="gxh", bufs=3))
                gxt_p = phw.enter_context(
                    tc.tile_pool(name="gxt", bufs=2, space="PSUM")
                )
                gxts_p = phw.enter_context(tc.tile_pool(name="gxts", bufs=4))
                gp_p = phw.enter_context(
                    tc.tile_pool(name="gp", bufs=2, space="PSUM")
                )
                for g in range(NG):
                    gxin6 = gxin_p.tile([D, 6, 256], bf16, tag="gxin2")
                    nc.sync.dma_start(
                        out=gxin6.rearrange("p b c -> p (b c)"),
                        in_=x_gate[g, :, :],
                    )
                    gxh4 = gxh_p.tile([D, 4, 3, D], bf16, tag="gxh")
                    for s in range(4):
                        for t in range(3):
                            blk, half = ST2BLK[(s, t)]
                            nc.gpsimd.tensor_scalar(
                                out=gxh4[:, s, t],
                                in0=gxin6[:, blk, half : 256 : 2],
                                scalar1=gst_st[:, g, blk,
                                               1 + 3 * half : 2 + 3 * half],
                                scalar2=gst_st[:, g, blk,
                                               2 + 3 * half : 3 + 3 * half],
                                op0=sub,
                                op1=mult,
                            )
                    for s in range(4):
                        y = 4 * g + s
                        gxt = gxt_p.tile([D, L], bf16, tag="gxt")
                        for t in range(3):
                            nc.tensor.transpose(
                                out=gxt[:, t * D : (t + 1) * D],
                                in_=gxh4[:, s, t],
                                identity=id_bf,
                            )
                        gxts = gxts_p.tile([D, L], bf16, tag="gxts")
                        if s % 2 == 0:
                            nc.vector.tensor_copy(out=gxts, in_=gxt)
                        else:
                            nc.scalar.activation(out=gxts, in_=gxt, func=Identity)
                        gp = gp_p.tile([D, L], f32, tag="gp")
                        nc.tensor.matmul(gp, wg_sb, gxts, start=True, stop=True)
                        # pre-activation (Wg xhat + cg) stored; Sigmoid later
                        nc.scalar.activation(
                            out=g_st[:, y, :], in_=gp, func=Identity,
                            bias=cg_sb, scale=1.0,
                        )

            # ---------------- collective 2: AllGather attn ----------------
            nc.gpsimd.collective_compute(
                "AllGather",
                mybir.AluOpType.bypass,
                replica_groups=groups,
                ins=[attn_bounce.ap().opt()],
                outs=[attn_full.ap().opt()],
            )

            # -------- window work C: bulk Sigmoid on gate pre-acts ---------
            for y in range(R):
                nc.scalar.activation(
                    out=g_st[:, y, :], in_=g_st[:, y, :], func=Sigmoid
                )

            # ---------------- phase 3: attn^T, AV, gate, Wo ----------------
            with ExitStack() as ph4:
                at_in_p = ph4.enter_context(tc.tile_pool(name="atin", bufs=3))
                at_st = ph4.enter_context(tc.tile_pool(name="atst", bufs=1))
                attnT = at_st.tile([D, H, 3, L], bf16)  # [j, h, jc, x]
                at_in = [at_in_p.tile([D, H, L], bf16, tag="atin", name=f"at_in_{i}") for i in range(3)]
                for ic in range(3):
                    asrc = bass.AP(
                        tensor=attn_full.ap().tensor,
                        offset=ic * H * L,
                        ap=[[3 * H * L, D], [1, H * L]],
                    )
                    nc.scalar.dma_start(
                        out=at_in[ic].rearrange("p h l -> p (h l)"), in_=asrc
                    )
                for h in range(H):
                    for jc in range(3):
                        for ic in range(3):
                            nc.sync.dma_start_transpose(
                                out=attnT[:, h, jc, ic * D : (ic + 1) * D],
                                in_=at_in[ic][:, h, jc * D : (jc + 1) * D],
                            )

                av_p = ph4.enter_context(
                    tc.tile_pool(name="av", bufs=4, space="PSUM")
                )
                gt_p = ph4.enter_context(tc.tile_pool(name="gt", bufs=4))
                wo_ps = ph4.enter_context(
                    tc.tile_pool(name="wops", bufs=3, space="PSUM")
                )
                os_p = ph4.enter_context(tc.tile_pool(name="osb", bufs=4))
                for y in range(R):
                    pav = av_p.tile([D, L], f32, tag="av")
                    for h in range(H):
                        for jc in range(3):
                            nc.tensor.matmul(
                                pav[h * DH : (h + 1) * DH, :],
                                v_st[:, jc, y, h * DH : (h + 1) * DH],
                                attnT[:, h, jc, :],
                                start=(jc == 0),
                                stop=(jc == 2),
                                tile_position=(0, h * DH),
                            )
                    gated = gt_p.tile([D, L], bf16, tag="gt")
                    nc.vector.scalar_tensor_tensor(
                        out=gated,
                        in0=pav,
                        scalar=cv_sb,
                        in1=g_st[:, y, :],
                        op0=add,
                        op1=mult,
                    )
                    pwo = wo_ps.tile([D, 3, D], f32, tag="wops")
                    for xc in range(3):
                        nc.tensor.matmul(
                            pwo[:, xc, :],
                            gated[:, xc * D : (xc + 1) * D],
                            wo_sb,
                            start=True,
                            stop=True,
                        )
                    if y % 4 == 0:
                        osb4 = os_p.tile([D, 4, 3, D], bf16, tag="osb",
                                         name=f"osb4_{y}")
                    # fused psum->sbuf eviction with +bo
                    nc.vector.tensor_add(
                        out=osb4[:, y % 4], in0=pwo, in1=bo_bc3
                    )
                    if y % 4 == 3:
                        nc.sync.dma_start(
                            out=out_p[y // 4, :, :],
                            in_=osb4.rearrange("p s t d -> p (s t d)"),
                        )

    nc.compile()
    return nc


def _prep_host(inputs):
    """Host-side: shard inputs, fold LN scale/bias + constants into weights."""
    f32 = np.float32
    bf = ml_dtypes.bfloat16
    pair = np.ascontiguousarray(np.asarray(inputs["pair"], f32)[0])
    bias = np.ascontiguousarray(np.asarray(inputs["bias"], f32)[0])
    ln_scale = np.asarray(inputs["ln_scale"], f32)
    ln_bias = np.asarray(inputs["ln_bias"], f32)
    Wq = np.asarray(inputs["Wq"], f32)
    Wk = np.asarray(inputs["Wk"], f32)
    Wv = np.asarray(inputs["Wv"], f32)
    Wb = np.asarray(inputs["Wb"], f32)
    Wg = np.asarray(inputs["Wg"], f32)
    bg = np.asarray(inputs["bg"], f32)
    Wo = np.asarray(inputs["Wo"], f32)
    bo = np.asarray(inputs["bo"], f32)

    RT_D = math.sqrt(D)  # istd is computed unscaled on-chip; fold sqrt(128)
    Wq_eff = Wq * ln_scale[None, :] * SCALING * RT_D
    Wk_eff = Wk * ln_scale[None, :] * KSCALE * RT_D
    cq = (Wq @ ln_bias) * SCALING
    ck = (Wk @ ln_bias) * KSCALE

    def scat(W_eff):
        w = np.zeros((16, D, D), f32)
        for h in range(H):
            for s in range(4):
                for kk in range(DH):
                    w[h * 4 + s, :, kk * 4 + s] = W_eff[h * DH + kk, :]
        return w.astype(bf)

    wq_scat = scat(Wq_eff)
    wk_scat = scat(Wk_eff)
    cq_sh = np.zeros((H, D), f32)
    ck_sh = np.zeros((H, D), f32)
    for h in range(H):
        for s in range(4):
            for kk in range(DH):
                cq_sh[h, kk * 4 + s] = cq[h * DH + kk]
                ck_sh[h, kk * 4 + s] = ck[h * DH + kk]

    pair_bf = pair.astype(bf)
    bias_bf = bias.astype(bf)
    shared = {
        "wq_scat": wq_scat,
        "wk_scat": wk_scat,
        "wv_t": (Wv * ln_scale[None, :] * RT_D).T.astype(bf).copy(),
        "wg_t": (Wg * ln_scale[None, :] * RT_D).T.astype(bf).copy(),
        "wo_t": Wo.T.astype(bf).copy(),
        "wb_t": Wb.T.astype(bf).copy(),
        "cq_sh": cq_sh,
        "ck_sh": ck_sh,
        "cg_v": (Wg @ ln_bias + bg).astype(f32),
        "cv_v": (Wv @ ln_bias).astype(f32),
        "bo_v": bo.astype(f32),
    }

    # pair-interleaved 6-block relayout: [48,384,128] -> [NG,128,6*256]
    BLK = [(0, 0, 0), (0, 1, 0), (0, 2, 1), (1, 2, 1), (1, 0, 2), (1, 1, 2),
           (2, 0, 3), (2, 1, 3), (2, 2, 4), (3, 2, 4), (3, 0, 5), (3, 1, 5)]

    def relayout6(x48):
        tmp = x48.reshape(NG, 4, 128, 3, D)  # [g, s, p, t, d]
        arr = np.empty((NG, 128, 6, D, 2), x48.dtype)
        half_used = [0] * 6
        for s, t, blk in BLK:
            arr[:, :, blk, :, half_used[blk]] = tmp[:, s, :, t, :]
            half_used[blk] += 1
        return np.ascontiguousarray(arr.reshape(NG, 128, 6 * 256))

    def relayout_std(x48):
        tmp = x48.reshape(NG, 4, 128, 3, D)  # [g, s, p, t, d]
        return np.ascontiguousarray(
            tmp.transpose(0, 2, 1, 3, 4).reshape(NG, 128, 4 * 3 * D)
        )

    in_maps = []
    for c in range(NCORES):
        sl = slice(c * R, (c + 1) * R)
        m = dict(shared)
        m["x_qkv"] = relayout6(
            np.ascontiguousarray(pair_bf[:, sl, :].transpose(1, 0, 2))
        )
        m["x_gate"] = relayout6(np.ascontiguousarray(pair_bf[sl, :, :]))
        m["bias_c"] = relayout_std(np.ascontiguousarray(bias_bf[sl, :, :]))
        in_maps.append(m)
    return in_maps


def kernel(**inputs):
    import os
    from concourse.bass_utils import run_bass_kernel_spmd

    in_maps = _prep_host(inputs)
    if "nc" not in _CACHE:
        _CACHE["nc"] = _build_graph()
    nc = _CACHE["nc"]
    kw = {}
    if os.environ.get("BAX_TRACE"):
        kw = dict(trace=True, tmpdir=os.environ.get("BAX_TRACE_DIR") or None)
    res = run_bass_kernel_spmd(nc, in_maps, list(range(NCORES)), **kw)
    _CACHE["last_result"] = res
    out = np.zeros((1, L, L, D), np.float32)
    for c in range(NCORES):
        # [NG,128,(s t d)] bf16 -> [R, L, D] f32 (x = 3p + t)
        o = np.asarray(res.results[c]["out"], np.float32)
        o = o.reshape(NG, 128, 4, 3, D).transpose(0, 2, 1, 3, 4)
        out[0, c * R : (c + 1) * R, :, :] = o.reshape(R, L, D)
    return out


if __name__ == "__main__":
    nc = _build_graph()
    print("graph built ok")
